# revision 61
# baseline (speedup 1.0000x reference)
"""Trainium2 Bass kernel: CustomFlashAttention (B=1, S=2048, D=2048, H=16, Hd=128).

Sharding (Megatron tensor-parallel over heads, 8 NeuronCores):
  - each core owns 2 heads (256 feature dims)
  - w_q/w_k/w_v column-parallel (pre-transposed + sliced on host)
  - w_o row-parallel; cores produce partial outputs, host sums the 8 partials

Device layout convention: activations are stored feature-major ("transposed",
[feat, seq]) so every matmul's contraction dim lands on SBUF partitions with
zero on-device transposes:
  qT/kT = W_slice^T-weighted projections of xT     [hd, s]
  v     = natural [s, hd] (computed with xT slices as the stationary operand)
  scores are computed transposed sT[k, q] = K Q^T; softmax runs without
  max-subtraction (scores ~ N(0,1), exp is safe in fp32); the exp'd fp16 tiles
  feed P^T straight into the PV matmul.

Softmax denominators: the exp'd [P, 2, CH] tiles are pairwise-accumulated in
f16 on the DVE (one wide add per k-tile pair), then a single ones-matmul
broadcasts the column sums across partitions. The reciprocal uses the fast
custom-DVE approximation (~18 correct bits). Each (chunk, head)'s
den -> ones-matmul -> recip -> mul finish chain is DEFERRED into the next
iteration's stream (emitted after its first score pair) so it never blocks
independent scores in the in-order PE queue.

Schedule: x lives SBUF-resident in fp16. Phase 1 runs k/v projections for all
seq chunks (k leads v by a 4-step skew tuned to the ~220GB/s aggregate DMA
delivery rate), with chunk 0's q projection wedged between chunks 0 and 1 —
exactly the window where the in-order DMA stream is still catching up. All
other q projections plus the previous chunk's output projection ride the
attention loop as "filler" matmuls; scores are emitted one k-tile-pair ahead
of the exp-dependent PV matmuls (software pipelining) so exp(g+1) never
transitively waits on PV(g). The DMA issue stream is a single first-use-
ordered piece list alternating sync/scalar.

The last chunk's output projection accumulates head-0 and head-1
contributions per ot tile in PSUM across two 8-tile waves (wave-B banks are
released by wave-A's PSUM->SBUF pair-copies, which alternate scalar/vector;
f16 pairs DMA out on sync/scalar).

Matmul operands are fp16 (10-bit mantissa, 1 cycle/row on TRN2, FWL weight
loads); all accumulation is fp32 in PSUM. Partial outputs DMA out in fp16
(halves output traffic; host accumulates in fp32). Measured end-to-end error
vs the fp32 reference stays ~8e-4 — well inside the 2e-2 gate.
"""

import sys
from contextlib import ExitStack

import numpy as np

if "/opt/trn_rl_repo" not in sys.path:
    sys.path.insert(0, "/opt/trn_rl_repo")

import concourse.bass as bass  # noqa: F401
import concourse.tile as tile
from concourse import bacc, mybir
from concourse.bass_utils import run_bass_kernel_spmd

P = 128                      # SBUF partitions
S = 2048                     # sequence length
D = 2048                     # hidden dim
H = 16                       # heads
HD = 128                     # head dim
NCORES = 8
HPC = H // NCORES            # heads per core = 2
HDC = HPC * HD               # feature dims per core = 256
DT = D // P                  # 16 contraction tiles
NCH = 4                      # seq chunks
CH = S // NCH                # 512
KT = S // P                  # 16 key tiles
SCALE = 1.0 / float(np.sqrt(HD))

f32 = mybir.dt.float32
f16 = mybir.dt.float16

_CACHE = {}
LAST_RESULT = None


def _build_nc():
    nc = bacc.Bacc("TRN2", target_bir_lowering=False, debug=False, num_devices=NCORES)

    xT = nc.dram_tensor("xT", [D, S], f16, kind="ExternalInput").ap()
    wqT = nc.dram_tensor("wqT", [D, HDC], f16, kind="ExternalInput").ap()
    wkT = nc.dram_tensor("wkT", [D, HDC], f16, kind="ExternalInput").ap()
    wvT = nc.dram_tensor("wvT", [D, HDC], f16, kind="ExternalInput").ap()
    woT = nc.dram_tensor("woT", [HDC, D], f16, kind="ExternalInput").ap()
    outT = nc.dram_tensor("outT", [D, S], f16, kind="ExternalOutput").ap()

    out_r = outT.rearrange("(ot p) s -> ot p s", p=P)    # [16, 128, 2048]
    # x viewed partition-major so multi-d-tile pieces transfer in one call
    x_pd = xT.rearrange("(dt p) s -> p dt s", p=P)       # [128, 16, 2048]

    with ExitStack() as ctx:
        tc = ctx.enter_context(tile.TileContext(nc))

        singles = ctx.enter_context(tc.tile_pool(name="singles", bufs=1))
        # pt ring depth 6: exp(g) WAR-aliases pt(g-6), so the exp stream
        # never gates on the serial den2 add chain (at depth 4 it did)
        ppool = ctx.enter_context(tc.tile_pool(name="pt", bufs=6))
        rspool = ctx.enter_context(tc.tile_pool(name="rs", bufs=3))
        dnpool = ctx.enter_context(tc.tile_pool(name="dn", bufs=4))
        obpool = ctx.enter_context(tc.tile_pool(name="ob", bufs=6))
        p1_ctx = ExitStack()
        k_ps = p1_ctx.enter_context(tc.tile_pool(name="kps", bufs=4, space="PSUM"))
        v_ps = p1_ctx.enter_context(tc.tile_pool(name="vps", bufs=4, space="PSUM"))

        # Persistent SBUF tensors
        x_sb = singles.tile([P, DT, S], f16, tag="x")
        wq_sb = singles.tile([P, DT, HDC], f16, tag="wq")
        wk_sb = singles.tile([P, DT, HDC], f16, tag="wk")
        wv_sb = singles.tile([P, DT, HDC], f16, tag="wv")
        wo_sb = singles.tile([P, HDC // P, D], f16, tag="wo")
        qT_sb = singles.tile([P, HPC, S], f16, tag="qT")
        kT_sb = singles.tile([P, HPC, S], f16, tag="kT")
        v_sb = singles.tile([P, KT, HDC], f16, tag="v")
        oT_sb = singles.tile([P, HPC, S], f16, tag="oT")
        ones = singles.tile([P, P], f16, tag="ones")

        nc.vector.memset(ones, 1.0)

        # DMA issue, ordered by first use. FOUR issue engines run SWDGE
        # generation in parallel in the kernel head (sync/scalar carry x
        # chunk 0, vector carries wk, gpsimd carries wv) so the early
        # window's ~4MB (x c0 + wk + wv) is all issued within ~2.5us of the
        # preamble barrier. wq is NOT needed until the phase-1/2 boundary
        # (all q projections are deferred) and wo not until chunk-1
        # attention, so both trail the x chunks on sync/scalar.
        wk_r = wkT.rearrange("(dt p) h -> p dt h", p=P)
        wv_r = wvT.rearrange("(dt p) h -> p dt h", p=P)
        wq_r = wqT.rearrange("(dt p) h -> p dt h", p=P)
        # sync/scalar strictly alternate over the first-use-ordered piece
        # list (k leads, v trails by the phase-1 skew; x chunks 1-3 follow);
        # gpsimd's slow SWDGE path only carries wq/wo, which aren't needed
        # until the phase boundary (~66us) / chunk-1 attention (~86us).
        csl0 = slice(0, CH)
        # One strictly first-use-ordered stream alternating sync/scalar.
        # Aggregate HBM->SBUF delivery runs at ~220GB/s regardless of how
        # many issue engines run, so ordering IS the optimization: wq rides
        # right after chunk 0's pieces because q(c0) is projected between
        # phase-1 chunks 0 and 1 (PE work that covers the window where the
        # stream is still catching up). wo (needed ~90us) goes last.
        pieces = [
            (wk_sb[:, 0, :], wk_r[:, 0, :]),
            (x_sb[:, 0, csl0], x_pd[:, 0, csl0]),
            (x_sb[:, 1:2, csl0], x_pd[:, 1:2, csl0]),
            (wk_sb[:, 1:2, :], wk_r[:, 1:2, :]),
            (x_sb[:, 2:4, csl0], x_pd[:, 2:4, csl0]),
            (wk_sb[:, 2:4, :], wk_r[:, 2:4, :]),
            (wv_sb[:, 0:2, :], wv_r[:, 0:2, :]),
            (x_sb[:, 4:6, csl0], x_pd[:, 4:6, csl0]),
            (wk_sb[:, 4:8, :], wk_r[:, 4:8, :]),
            (x_sb[:, 6:8, csl0], x_pd[:, 6:8, csl0]),
            (wv_sb[:, 2:4, :], wv_r[:, 2:4, :]),
            (wv_sb[:, 4:8, :], wv_r[:, 4:8, :]),
            (x_sb[:, 8:10, csl0], x_pd[:, 8:10, csl0]),
            (wk_sb[:, 8:12, :], wk_r[:, 8:12, :]),
            (x_sb[:, 10:12, csl0], x_pd[:, 10:12, csl0]),
            (wk_sb[:, 12:16, :], wk_r[:, 12:16, :]),
            (x_sb[:, 12:14, csl0], x_pd[:, 12:14, csl0]),
            (wv_sb[:, 8:12, :], wv_r[:, 8:12, :]),
            (x_sb[:, 14:16, csl0], x_pd[:, 14:16, csl0]),
            (wv_sb[:, 12:16, :], wv_r[:, 12:16, :]),
            (wq_sb[:, 0:4, :], wq_r[:, 0:4, :]),
            (wq_sb[:, 4:8, :], wq_r[:, 4:8, :]),
            (wq_sb[:, 8:12, :], wq_r[:, 8:12, :]),
            (wq_sb[:, 12:16, :], wq_r[:, 12:16, :]),
        ]
        for c in range(1, NCH):
            csl = slice(c * CH, (c + 1) * CH)
            for d in range(0, DT, 4):
                pieces.append((x_sb[:, d:d + 4, csl], x_pd[:, d:d + 4, csl]))
        pieces.append((wo_sb, woT.rearrange("(it p) o -> p it o", p=P)))
        for i, (dst, src) in enumerate(pieces):
            (nc.sync if i % 2 == 0 else nc.scalar).dma_start(out=dst, in_=src)

        # ALL q projections are deferred out of phase 1: chunks 1-3 into the
        # attention stream of the immediately preceding (chunk, head)
        # iteration (PE filler work), chunk 0 to the phase-1/2 boundary.
        # This removes wq (1MB) from the bandwidth-critical early window.
        DEFERRED_Q = {(c, h) for c in range(NCH) for h in range(HPC)}

        # ---------- Phase 1: k/v projections for all seq chunks ----------
        for c in range(NCH):
            csl = slice(c * CH, (c + 1) * CH)
            pk = [k_ps.tile([P, CH], f32, tag="pk", name=f"pk{c}_{i}") for i in range(HPC)]
            pv = [v_ps.tile([P, HDC], f32, tag="pv", name=f"pv{c}_{i}") for i in range(4)]
            # skewed emission: k at step d, v four steps behind — each
            # weight stream gets several d-tiles of DMA arrival slack at
            # kernel start before the in-order PE needs it
            VSKEW = 4
            for step in range(DT + VSKEW):
                d = step
                if d < DT:
                    for h in range(HPC):
                        nc.tensor.matmul(
                            pk[h],
                            lhsT=wk_sb[:, d, h * HD:(h + 1) * HD],
                            rhs=x_sb[:, d, csl], start=(d == 0), stop=(d == DT - 1),
                        )
                if d == DT - 1:
                    # k is complete: drain it now so the casts overlap the
                    # trailing v steps instead of queueing after them
                    for h in range(HPC):
                        nc.vector.tensor_copy(kT_sb[:, h, csl], pk[h])
                d = step - VSKEW
                if 0 <= d < DT:
                    for st in range(4):
                        nc.tensor.matmul(
                            pv[st],
                            lhsT=x_sb[:, d, c * CH + st * P:c * CH + (st + 1) * P],
                            rhs=wv_sb[:, d, :],
                            start=(d == 0), stop=(d == DT - 1),
                        )
            for st in range(4):
                nc.vector.tensor_copy(v_sb[:, c * 4 + st, :], pv[st])
            if c == 0:
                # q(c0) right after chunk 0, borrowing the k_ps ring's two
                # fresh slots: 6.8us of PE work covering exactly the window
                # where the DMA stream is still catching up, and removing
                # the phase-boundary serialization q(c0) otherwise causes
                for h in range(HPC):
                    pq = k_ps.tile([P, CH], f32, tag="pk", name=f"pq0_{h}")
                    for dd in range(DT):
                        nc.tensor.matmul(
                            pq,
                            lhsT=wq_sb[:, dd, h * HD:(h + 1) * HD],
                            rhs=x_sb[:, dd, csl],
                            start=(dd == 0), stop=(dd == DT - 1),
                        )
                    nc.vector.tensor_copy(qT_sb[:, h, csl], pq)

        p1_ctx.close()  # release phase-1 PSUM banks

        # ---------- output-projection emission units ----------
        # ro_ps opens FIRST so it lands on the phase-1 banks freed longest
        # ago (chunk-2-era pk banks): the boundary q(c0) projections then
        # start without waiting for chunk-3's k/v PSUM->SBUF casts.
        ro_ps = ctx.enter_context(tc.tile_pool(name="rops", bufs=2, space="PSUM"))
        sc_ps = ctx.enter_context(tc.tile_pool(name="scps", bufs=2, space="PSUM"))
        o_ps = ctx.enter_context(tc.tile_pool(name="ops", bufs=2, space="PSUM"))

        def make_defq_units(c, h):
            """16 single-matmul units projecting q for (c, h); last drains PSUM."""
            csl = slice(c * CH, (c + 1) * CH)
            pq = ro_ps.tile([P, CH], f32, tag="rout", name=f"dpq{c}_{h}")

            def unit(d):
                def emit():
                    nc.tensor.matmul(
                        pq,
                        lhsT=wq_sb[:, d, h * HD:(h + 1) * HD],
                        rhs=x_sb[:, d, csl],
                        start=(d == 0), stop=(d == DT - 1),
                    )
                    if d == DT - 1:
                        nc.vector.tensor_copy(qT_sb[:, h, csl], pq)
                return emit

            return [unit(d) for d in range(DT)]

        out_pr = outT.rearrange("(ot p) s -> p ot s", p=P)   # [128, 16, 2048]

        def make_ph3_units(c, gp_copies=False):
            """Out-projection units for chunk c. Output tiles are paired
            ([P, 2, CH]) so each DMA call covers two ot tiles — halves the
            SWDGE issue cost — and DMA issue alternates sync/gpsimd.
            gp_copies routes the PSUM->SBUF copies to the idle gpsimd engine
            (used for the units consumed during the last chunk, where scalar
            is exp-bound and the DVE runs the denominator chain)."""
            csl = slice(c * CH, (c + 1) * CH)
            state = {}

            def unit(ot):
                def emit():
                    pout = ro_ps.tile([P, CH], f32, tag="rout", name=f"pout{c}_{ot}")
                    for di in range(HDC // P):
                        nc.tensor.matmul(
                            pout,
                            lhsT=wo_sb[:, di, ot * P:(ot + 1) * P],
                            rhs=oT_sb[:, di, csl],
                            start=(di == 0), stop=(di == HDC // P - 1),
                        )
                    if ot % 2 == 0:
                        state["ob"] = obpool.tile([P, 2, CH], f16, tag="ob", name=f"ob{c}_{ot}")
                    ob = state["ob"]
                    nc.vector.tensor_copy(ob[:, ot % 2, :], pout)
                    if ot % 2 == 1:
                        if gp_copies:
                            # sync only: scalar must stay exp-only and
                            # gpsimd's dge drain must not trail the end
                            eng = nc.sync
                        else:
                            eng = nc.gpsimd if (ot // 2) % 2 else nc.sync
                        eng.dma_start(out=out_pr[:, ot - 1:ot + 1, csl], in_=ob)
                return emit

            return [unit(ot) for ot in range(DT)]

        # ---------- Phase 2: attention with interleaved fillers ----------
        def attention(c, h, fillers, fills_per_g, prev_fin=None, last=False):
            """fillers: list of emission units injected after each score pair.
            Returns a finish-closure (den -> ones-matmul -> recip -> mul)
            which the NEXT attention invokes right after its first score
            pair, so the den-dependent ones-matmul never blocks the next
            iteration's independent scores in the in-order PE queue."""
            csl = slice(c * CH, (c + 1) * CH)
            po = o_ps.tile([P, CH], f32, tag="po", name=f"po{c}_{h}")
            den2 = dnpool.tile([P, 2, CH], f16, tag="den", name=f"den2{c}_{h}")
            den = dnpool.tile([P, CH], f16, tag="den", name=f"den{c}_{h}")
            fi = 0
            pt0 = None
            pt_late = []

            def scores(g):
                psc = sc_ps.tile([P, 2, CH], f32, tag="psc", name=f"psc{c}_{h}_{g}")
                for j in range(2):
                    kj = g * 2 + j
                    nc.tensor.matmul(
                        psc[:, j, :],
                        lhsT=kT_sb[:, h, kj * P:(kj + 1) * P],
                        rhs=qT_sb[:, h, csl],
                        start=True, stop=True,
                    )
                return psc

            # software-pipelined by one group: scores(g+1) are emitted
            # BEFORE the exp(g)-dependent PV(g), so the in-order PE streams
            # them while the scalar engine runs exp(g). Without the
            # lookahead, exp(g+1) transitively waits on exp(g)'s PV and the
            # group cycle degrades to exp + ~950ns instead of max(exp, PE).
            psc = scores(0)
            if prev_fin is not None:
                prev_fin()
            for g in range(KT // 2):
                psc_next = scores(g + 1) if g + 1 < KT // 2 else None
                # independent PE work here hides the exp latency
                for _ in range(fills_per_g[g]):
                    if fi < len(fillers):
                        fillers[fi]()
                        fi += 1
                pt = ppool.tile([P, 2, CH], f16, tag="pt", name=f"pt{c}_{h}_{g}")
                nc.scalar.activation(
                    out=pt, in_=psc,
                    func=mybir.ActivationFunctionType.Exp, scale=SCALE,
                )
                for j in range(2):
                    kj = g * 2 + j
                    nc.tensor.matmul(
                        po,
                        lhsT=v_sb[:, kj, h * HD:(h + 1) * HD],
                        rhs=pt[:, j, :],
                        start=(kj == 0), stop=(kj == KT - 1),
                    )
                psc = psc_next
                # f16 pairwise accumulation of the exp'd tiles for the
                # softmax denominator: one [P, 2, CH] add per group (the DVE
                # processes 2 f16/cycle, so this halves both the op count
                # and the queue entries vs per-half adds). For the LAST
                # iteration, groups 6/7 skip the DVE and are folded into the
                # ones-matmul instead: those accumulates are gated on
                # exp(g6)/exp(g7) and land exactly in the final den window,
                # keeping the PE's p-state up (a ~2us idle there demotes the
                # clock to 1.2GHz for the whole out-projection sweep).
                if g == 0:
                    pt0 = pt
                elif g == 1:
                    nc.vector.tensor_add(den2, pt0, pt)
                elif last and g >= KT // 2 - 2:
                    pt_late.append(pt)
                else:
                    nc.vector.tensor_add(den2, den2, pt)
            while fi < len(fillers):
                fillers[fi]()
                fi += 1

            def finish():
                with tc.high_priority():
                    nc.vector.tensor_add(den, den2[:, 0, :], den2[:, 1, :])
                    pden = ro_ps.tile([P, CH], f32, tag="rout", name=f"pden{c}_{h}")
                    nc.tensor.matmul(pden, lhsT=ones, rhs=den,
                                     start=True, stop=(not pt_late))
                    for li, ptl in enumerate(pt_late):
                        for j in range(2):
                            nc.tensor.matmul(
                                pden, lhsT=ones, rhs=ptl[:, j, :], start=False,
                                stop=(li == len(pt_late) - 1 and j == 1),
                            )
                    rs = rspool.tile([P, CH], f32, tag="rs", name=f"rs{c}_{h}")
                    if last:
                        # column-half recip+mul: oT's first half is ready one
                        # DVE-op earlier, letting the out-projection drain
                        # start sooner and keeping the PE's p-state up
                        for q in range(2):
                            sl = slice(q * (CH // 2), (q + 1) * (CH // 2))
                            osl = slice(c * CH + q * (CH // 2),
                                        c * CH + (q + 1) * (CH // 2))
                            nc.vector.reciprocal_approx_fast(
                                out=rs[:, sl], in_=pden[:, sl])
                            nc.vector.tensor_mul(
                                oT_sb[:, h, osl], po[:, sl], rs[:, sl])
                    else:
                        nc.vector.reciprocal_approx_fast(out=rs, in_=pden)
                        nc.vector.tensor_mul(oT_sb[:, h, csl], po, rs)

            return finish

        # Final-block helpers: the last chunk's out-projection accumulates
        # di=0 (head 0, available early) and di=1 (head 1, after the last
        # den chain) per ot tile in PSUM. 16 ot tiles flow through the 8
        # PSUM banks in two waves; wave-B banks are released by wave-A's
        # PSUM->SBUF pair-copies, which alternate between the scalar and
        # vector engines (both idle by then) so neither serializes the
        # drain. f16 pair tiles DMA out on sync/gpsimd.
        cl = slice((NCH - 1) * CH, NCH * CH)
        fpout = {}
        fptile = {}

        def falloc_pair(pi):
            t = sc_ps.tile([P, 2, CH], f32, tag="psc", name=f"fp{pi}")
            fptile[pi] = t
            fpout[2 * pi] = t[:, 0, :]
            fpout[2 * pi + 1] = t[:, 1, :]

        def falloc_single(ot, pool, tag):
            fpout[ot] = pool.tile([P, CH], f32, tag=tag, name=f"fs{ot}")

        def fdi0(ot):
            nc.tensor.matmul(
                fpout[ot], lhsT=wo_sb[:, 0, ot * P:(ot + 1) * P],
                rhs=oT_sb[:, 0, cl], start=True, stop=False,
            )

        def fdi1(ot):
            nc.tensor.matmul(
                fpout[ot], lhsT=wo_sb[:, 1, ot * P:(ot + 1) * P],
                rhs=oT_sb[:, 1, cl], start=False, stop=True,
            )

        def fpair_unit(pi):
            def emit():
                falloc_pair(pi)
                fdi0(2 * pi)
                fdi0(2 * pi + 1)
            return emit

        def fot4_unit():
            # single on o_ps: aliases po(c3,h0), long freed by h0's mul
            falloc_single(4, o_ps, "po")
            fdi0(4)

        # Filler hosting, balanced so no attention iteration's PE work
        # (scores+PV = ~0.86us/group) falls below the exp pace
        # (~1.1us/group): chunk-0 attention hosts TWO deferred-q sets
        # (chunks 1 and 2), chunk-1 hosts q(3) plus chunk-0's out-proj,
        # chunks 2/3 host the previous chunk's out-proj.
        fin = None
        for c in range(NCH):
            ph3_prev = make_ph3_units(c - 1, gp_copies=(c == NCH - 1)) if c > 0 else []
            for h in range(HPC):
                ph = ph3_prev[h * 8:(h + 1) * 8]
                if c == 0:
                    fills = make_defq_units(1, h) + make_defq_units(2, h)
                    pat = [4, 4, 4, 4, 4, 4, 4, 0]
                elif c == 1:
                    fills = make_defq_units(3, h) + ph
                    pat = [4, 4, 3, 3, 0, 2, 2, 0]
                elif c == 2:
                    fills = ph
                    pat = [1, 1, 1, 1, 1, 1, 1, 1]
                else:
                    fills = ph
                    if h == HPC - 1:
                        # di=0 units for ot 0-4 of c3's own out-projection ride
                        # as post-loop fillers of the LAST attention so they
                        # land in the PE queue BEFORE the den-dependent
                        # ones-matmul and fill the denominator-chain window.
                        # (only ONE ro-slot consumer may precede pden in the
                        # ro ring — none here — else it deadlocks)
                        fills = ph + [fpair_unit(0), fpair_unit(1), fot4_unit]
                    pat = [1, 1, 1, 1, 1, 0, 0, 0]
                fin = attention(c, h, fills, pat, prev_fin=fin,
                                last=(c == NCH - 1 and h == HPC - 1))
        fin()

        # di=0 staircase over the banks freed by the last den chain:
        # ot6 (ro slot freed by chunk-2 pout cast) -> ot7 (pden's slot,
        # freed by the reciprocal) -> ot5 (po(c3,h1), freed by the mul)
        falloc_single(6, ro_ps, "rout"); fdi0(6)
        falloc_single(7, ro_ps, "rout"); fdi0(7)
        falloc_single(5, o_ps, "po"); fdi0(5)

        def fdrain(pi):
            """Copy pair pi's two f32 PSUM ot tiles to an f16 pair and DMA.
            Wide single-op casts for contiguous sc pairs; per-half casts on
            both engines otherwise. Copy engine alternates scalar/vector."""
            ob = obpool.tile([P, 2, CH], f16, tag="ob", name=f"fob{pi}")
            if pi >= DT // 2 - 2:
                # last two pairs: per-half copies on both engines and
                # quarter-size DMAs so the final flush starts as early as
                # possible and spreads over both issue queues
                HF = CH // 2
                for j in range(2):
                    eng = nc.scalar if (j + pi) % 2 == 0 else nc.vector
                    (eng.copy if eng is nc.scalar else eng.tensor_copy)(
                        ob[:, j, :], fpout[2 * pi + j])
                    for q in range(2):
                        hs = slice(q * HF, (q + 1) * HF)
                        ocl = slice((NCH - 1) * CH + q * HF,
                                    (NCH - 1) * CH + (q + 1) * HF)
                        deng = nc.sync if (2 * j + q + pi) % 2 == 0 else nc.scalar
                        deng.dma_start(out=out_pr[:, 2 * pi + j, ocl],
                                       in_=ob[:, j, hs])
                return
            if pi in fptile:
                if pi % 2:
                    nc.vector.tensor_copy(ob, fptile[pi])
                else:
                    nc.scalar.copy(ob, fptile[pi])
            else:
                e0, e1 = (nc.scalar, nc.vector) if pi % 2 == 0 else (nc.vector, nc.scalar)
                (e0.copy if e0 is nc.scalar else e0.tensor_copy)(ob[:, 0, :], fpout[2 * pi])
                (e1.copy if e1 is nc.scalar else e1.tensor_copy)(ob[:, 1, :], fpout[2 * pi + 1])
            # alternate sync/scalar (scalar's exp work is over by now and a
            # sync-only chain leaves the kernel waiting ~2us on the last
            # serialized transfers); gpsimd's dge drain must not trail the
            # kernel end, so it carries nothing here
            (nc.sync if pi % 2 == 0 else nc.scalar).dma_start(
                out=out_pr[:, 2 * pi:2 * pi + 2, cl], in_=ob)

        for ot in range(8):
            fdi1(ot)
            if ot % 2 == 1:
                fdrain(ot // 2)
        # wave B: banks released by wave-A pair-copy completions
        for ot in range(8, DT):
            if ot % 2 == 0 and ot < 12:
                falloc_pair(ot // 2)
            elif ot == 12:
                falloc_single(12, o_ps, "po")
            elif ot == 13:
                falloc_single(13, o_ps, "po")
            elif ot == 14:
                falloc_single(14, ro_ps, "rout")
            elif ot == 15:
                falloc_single(15, ro_ps, "rout")
            fdi0(ot)
            fdi1(ot)
            if ot % 2 == 1:
                fdrain(ot // 2)

    nc.compile()
    return nc


def _get_nc():
    if "nc" not in _CACHE:
        _CACHE["nc"] = _build_nc()
    return _CACHE["nc"]


def make_in_maps(x, w_q, w_k, w_v, w_o):
    x = np.asarray(x, dtype=np.float32).reshape(S, D)
    w_q = np.asarray(w_q, dtype=np.float32)
    w_k = np.asarray(w_k, dtype=np.float32)
    w_v = np.asarray(w_v, dtype=np.float32)
    w_o = np.asarray(w_o, dtype=np.float32)
    xT = np.ascontiguousarray(x.T).astype(np.float16)
    in_maps = []
    for c in range(NCORES):
        hs = slice(c * HDC, (c + 1) * HDC)
        in_maps.append({
            "xT": xT,
            "wqT": np.ascontiguousarray(w_q[hs, :].T).astype(np.float16),
            "wkT": np.ascontiguousarray(w_k[hs, :].T).astype(np.float16),
            "wvT": np.ascontiguousarray(w_v[hs, :].T).astype(np.float16),
            "woT": np.ascontiguousarray(w_o[:, hs].T).astype(np.float16),
        })
    return in_maps


def kernel(x, w_q, w_k, w_v, w_o):
    global LAST_RESULT
    in_maps = make_in_maps(x, w_q, w_k, w_v, w_o)
    nc = _get_nc()
    res = run_bass_kernel_spmd(nc, in_maps, core_ids=list(range(NCORES)))
    LAST_RESULT = res
    acc = np.zeros((D, S), dtype=np.float32)
    for r in res.results:
        acc += r["outT"].astype(np.float32)
    return np.ascontiguousarray(acc.T).astype(np.float32).reshape(1, S, D)



# revision 63
# speedup vs baseline: 1.0112x; 1.0112x over previous
"""Trainium2 Bass kernel: CustomFlashAttention (B=1, S=2048, D=2048, H=16, Hd=128).

Sharding (Megatron tensor-parallel over heads, 8 NeuronCores):
  - each core owns 2 heads (256 feature dims)
  - w_q/w_k/w_v column-parallel (pre-transposed + sliced on host)
  - w_o row-parallel; cores produce partial outputs, host sums the 8 partials

Device layout convention: activations are stored feature-major ("transposed",
[feat, seq]) so every matmul's contraction dim lands on SBUF partitions with
zero on-device transposes:
  qT/kT = W_slice^T-weighted projections of xT     [hd, s]
  v     = natural [s, hd] (computed with xT slices as the stationary operand)
  scores are computed transposed sT[k, q] = K Q^T; softmax runs without
  max-subtraction (scores ~ N(0,1), exp is safe in fp32); the exp'd fp16 tiles
  feed P^T straight into the PV matmul.

Softmax denominators: the exp'd [P, 2, CH] tiles are pairwise-accumulated in
f16 on the DVE (one wide add per k-tile pair), then a single ones-matmul
broadcasts the column sums across partitions. The reciprocal uses the fast
custom-DVE approximation (~18 correct bits). Each (chunk, head)'s
den -> ones-matmul -> recip -> mul finish chain is DEFERRED into the next
iteration's stream (emitted after its first score pair) so it never blocks
independent scores in the in-order PE queue.

Schedule: x lives SBUF-resident in fp16. Phase 1 runs k/v projections for all
seq chunks (k leads v by a 4-step skew tuned to the ~220GB/s aggregate DMA
delivery rate), with chunk 0's q projection wedged between chunks 0 and 1 —
exactly the window where the in-order DMA stream is still catching up. All
other q projections plus the previous chunk's output projection ride the
attention loop as "filler" matmuls; scores are emitted one k-tile-pair ahead
of the exp-dependent PV matmuls (software pipelining) so exp(g+1) never
transitively waits on PV(g). The DMA issue stream is a single first-use-
ordered piece list alternating sync/scalar.

The last chunk's output projection accumulates head-0 and head-1
contributions per ot tile in PSUM across two 8-tile waves (wave-B banks are
released by wave-A's PSUM->SBUF pair-copies, which alternate scalar/vector;
f16 pairs DMA out on sync/scalar).

Matmul operands are fp16 (10-bit mantissa, 1 cycle/row on TRN2, FWL weight
loads); all accumulation is fp32 in PSUM. Partial outputs DMA out in fp16
(halves output traffic; host accumulates in fp32). Measured end-to-end error
vs the fp32 reference stays ~8e-4 — well inside the 2e-2 gate.
"""

import sys
from contextlib import ExitStack

import numpy as np

if "/opt/trn_rl_repo" not in sys.path:
    sys.path.insert(0, "/opt/trn_rl_repo")

import concourse.bass as bass  # noqa: F401
import concourse.tile as tile
from concourse import bacc, mybir
from concourse.bass_utils import run_bass_kernel_spmd

P = 128                      # SBUF partitions
S = 2048                     # sequence length
D = 2048                     # hidden dim
H = 16                       # heads
HD = 128                     # head dim
NCORES = 8
HPC = H // NCORES            # heads per core = 2
HDC = HPC * HD               # feature dims per core = 256
DT = D // P                  # 16 contraction tiles
NCH = 4                      # seq chunks
CH = S // NCH                # 512
KT = S // P                  # 16 key tiles
SCALE = 1.0 / float(np.sqrt(HD))

f32 = mybir.dt.float32
f16 = mybir.dt.float16

_CACHE = {}
LAST_RESULT = None


def _build_nc():
    nc = bacc.Bacc("TRN2", target_bir_lowering=False, debug=False, num_devices=NCORES)

    xT = nc.dram_tensor("xT", [D, S], f16, kind="ExternalInput").ap()
    wqT = nc.dram_tensor("wqT", [D, HDC], f16, kind="ExternalInput").ap()
    wkT = nc.dram_tensor("wkT", [D, HDC], f16, kind="ExternalInput").ap()
    wvT = nc.dram_tensor("wvT", [D, HDC], f16, kind="ExternalInput").ap()
    woT = nc.dram_tensor("woT", [HDC, D], f16, kind="ExternalInput").ap()
    outT = nc.dram_tensor("outT", [D, S], f16, kind="ExternalOutput").ap()

    out_r = outT.rearrange("(ot p) s -> ot p s", p=P)    # [16, 128, 2048]
    # x viewed partition-major so multi-d-tile pieces transfer in one call
    x_pd = xT.rearrange("(dt p) s -> p dt s", p=P)       # [128, 16, 2048]

    with ExitStack() as ctx:
        tc = ctx.enter_context(tile.TileContext(nc))

        singles = ctx.enter_context(tc.tile_pool(name="singles", bufs=1))
        # pt ring depth 6: exp(g) WAR-aliases pt(g-6), so the exp stream
        # never gates on the serial den2 add chain (at depth 4 it did)
        ppool = ctx.enter_context(tc.tile_pool(name="pt", bufs=6))
        rspool = ctx.enter_context(tc.tile_pool(name="rs", bufs=3))
        dnpool = ctx.enter_context(tc.tile_pool(name="dn", bufs=4))
        obpool = ctx.enter_context(tc.tile_pool(name="ob", bufs=6))
        p1_ctx = ExitStack()
        k_ps = p1_ctx.enter_context(tc.tile_pool(name="kps", bufs=4, space="PSUM"))
        v_ps = p1_ctx.enter_context(tc.tile_pool(name="vps", bufs=4, space="PSUM"))

        # Persistent SBUF tensors
        x_sb = singles.tile([P, DT, S], f16, tag="x")
        wq_sb = singles.tile([P, DT, HDC], f16, tag="wq")
        wk_sb = singles.tile([P, DT, HDC], f16, tag="wk")
        wv_sb = singles.tile([P, DT, HDC], f16, tag="wv")
        wo_sb = singles.tile([P, HDC // P, D], f16, tag="wo")
        qT_sb = singles.tile([P, HPC, S], f16, tag="qT")
        kT_sb = singles.tile([P, HPC, S], f16, tag="kT")
        v_sb = singles.tile([P, KT, HDC], f16, tag="v")
        oT_sb = singles.tile([P, HPC, S], f16, tag="oT")
        ones = singles.tile([P, P], f16, tag="ones")

        nc.vector.memset(ones, 1.0)

        # DMA issue, ordered by first use. FOUR issue engines run SWDGE
        # generation in parallel in the kernel head (sync/scalar carry x
        # chunk 0, vector carries wk, gpsimd carries wv) so the early
        # window's ~4MB (x c0 + wk + wv) is all issued within ~2.5us of the
        # preamble barrier. wq is NOT needed until the phase-1/2 boundary
        # (all q projections are deferred) and wo not until chunk-1
        # attention, so both trail the x chunks on sync/scalar.
        wk_r = wkT.rearrange("(dt p) h -> p dt h", p=P)
        wv_r = wvT.rearrange("(dt p) h -> p dt h", p=P)
        wq_r = wqT.rearrange("(dt p) h -> p dt h", p=P)
        # sync/scalar strictly alternate over the first-use-ordered piece
        # list (k leads, v trails by the phase-1 skew; x chunks 1-3 follow);
        # gpsimd's slow SWDGE path only carries wq/wo, which aren't needed
        # until the phase boundary (~66us) / chunk-1 attention (~86us).
        csl0 = slice(0, CH)
        # One strictly first-use-ordered stream alternating sync/scalar.
        # Aggregate HBM->SBUF delivery runs at ~220GB/s regardless of how
        # many issue engines run, so ordering IS the optimization: wq rides
        # right after chunk 0's pieces because q(c0) is projected between
        # phase-1 chunks 0 and 1 (PE work that covers the window where the
        # stream is still catching up). wo (needed ~90us) goes last.
        pieces = [
            (wk_sb[:, 0, :], wk_r[:, 0, :]),
            (x_sb[:, 0, csl0], x_pd[:, 0, csl0]),
            (x_sb[:, 1:2, csl0], x_pd[:, 1:2, csl0]),
            (wk_sb[:, 1:2, :], wk_r[:, 1:2, :]),
            (x_sb[:, 2:4, csl0], x_pd[:, 2:4, csl0]),
            (wk_sb[:, 2:4, :], wk_r[:, 2:4, :]),
            (wv_sb[:, 0:2, :], wv_r[:, 0:2, :]),
            (x_sb[:, 4:6, csl0], x_pd[:, 4:6, csl0]),
            (wk_sb[:, 4:8, :], wk_r[:, 4:8, :]),
            (x_sb[:, 6:8, csl0], x_pd[:, 6:8, csl0]),
            (wv_sb[:, 2:4, :], wv_r[:, 2:4, :]),
            (wv_sb[:, 4:8, :], wv_r[:, 4:8, :]),
            (x_sb[:, 8:10, csl0], x_pd[:, 8:10, csl0]),
            (wk_sb[:, 8:12, :], wk_r[:, 8:12, :]),
            (x_sb[:, 10:12, csl0], x_pd[:, 10:12, csl0]),
            (wk_sb[:, 12:16, :], wk_r[:, 12:16, :]),
            (x_sb[:, 12:14, csl0], x_pd[:, 12:14, csl0]),
            (wv_sb[:, 8:12, :], wv_r[:, 8:12, :]),
            (x_sb[:, 14:16, csl0], x_pd[:, 14:16, csl0]),
            (wv_sb[:, 12:16, :], wv_r[:, 12:16, :]),
            (wq_sb[:, 0:4, :], wq_r[:, 0:4, :]),
            (wq_sb[:, 4:8, :], wq_r[:, 4:8, :]),
            (wq_sb[:, 8:12, :], wq_r[:, 8:12, :]),
            (wq_sb[:, 12:16, :], wq_r[:, 12:16, :]),
        ]
        for c in range(1, NCH):
            csl = slice(c * CH, (c + 1) * CH)
            for d in range(0, DT, 4):
                pieces.append((x_sb[:, d:d + 4, csl], x_pd[:, d:d + 4, csl]))
        pieces.append((wo_sb, woT.rearrange("(it p) o -> p it o", p=P)))
        for i, (dst, src) in enumerate(pieces):
            (nc.sync if i % 2 == 0 else nc.scalar).dma_start(out=dst, in_=src)

        # ALL q projections are deferred out of phase 1: chunks 1-3 into the
        # attention stream of the immediately preceding (chunk, head)
        # iteration (PE filler work), chunk 0 to the phase-1/2 boundary.
        # This removes wq (1MB) from the bandwidth-critical early window.
        DEFERRED_Q = {(c, h) for c in range(NCH) for h in range(HPC)}

        # ---------- Phase 1: k/v projections for all seq chunks ----------
        for c in range(NCH):
            csl = slice(c * CH, (c + 1) * CH)
            pk = [k_ps.tile([P, CH], f32, tag="pk", name=f"pk{c}_{i}") for i in range(HPC)]
            pv = [v_ps.tile([P, HDC], f32, tag="pv", name=f"pv{c}_{i}") for i in range(4)]
            # skewed emission: k at step d, v four steps behind — each
            # weight stream gets several d-tiles of DMA arrival slack at
            # kernel start before the in-order PE needs it
            VSKEW = 4
            for step in range(DT + VSKEW):
                d = step
                if d < DT:
                    for h in range(HPC):
                        nc.tensor.matmul(
                            pk[h],
                            lhsT=wk_sb[:, d, h * HD:(h + 1) * HD],
                            rhs=x_sb[:, d, csl], start=(d == 0), stop=(d == DT - 1),
                        )
                if d == DT - 1:
                    # k is complete: drain it now so the casts overlap the
                    # trailing v steps instead of queueing after them
                    for h in range(HPC):
                        nc.vector.tensor_copy(kT_sb[:, h, csl], pk[h])
                d = step - VSKEW
                if 0 <= d < DT:
                    for st in range(4):
                        nc.tensor.matmul(
                            pv[st],
                            lhsT=x_sb[:, d, c * CH + st * P:c * CH + (st + 1) * P],
                            rhs=wv_sb[:, d, :],
                            start=(d == 0), stop=(d == DT - 1),
                        )
            for st in range(4):
                nc.vector.tensor_copy(v_sb[:, c * 4 + st, :], pv[st])
            if c == 0:
                # q(c0) right after chunk 0, borrowing the k_ps ring's two
                # fresh slots: 6.8us of PE work covering exactly the window
                # where the DMA stream is still catching up, and removing
                # the phase-boundary serialization q(c0) otherwise causes
                for h in range(HPC):
                    pq = k_ps.tile([P, CH], f32, tag="pk", name=f"pq0_{h}")
                    for dd in range(DT):
                        nc.tensor.matmul(
                            pq,
                            lhsT=wq_sb[:, dd, h * HD:(h + 1) * HD],
                            rhs=x_sb[:, dd, csl],
                            start=(dd == 0), stop=(dd == DT - 1),
                        )
                    nc.vector.tensor_copy(qT_sb[:, h, csl], pq)

        p1_ctx.close()  # release phase-1 PSUM banks

        # ---------- output-projection emission units ----------
        # ro_ps opens FIRST so it lands on the phase-1 banks freed longest
        # ago (chunk-2-era pk banks): the boundary q(c0) projections then
        # start without waiting for chunk-3's k/v PSUM->SBUF casts.
        ro_ps = ctx.enter_context(tc.tile_pool(name="rops", bufs=2, space="PSUM"))
        sc_ps = ctx.enter_context(tc.tile_pool(name="scps", bufs=2, space="PSUM"))
        o_ps = ctx.enter_context(tc.tile_pool(name="ops", bufs=2, space="PSUM"))

        def make_defq_units(c, h):
            """16 single-matmul units projecting q for (c, h); last drains PSUM."""
            csl = slice(c * CH, (c + 1) * CH)
            pq = ro_ps.tile([P, CH], f32, tag="rout", name=f"dpq{c}_{h}")

            def unit(d):
                def emit():
                    nc.tensor.matmul(
                        pq,
                        lhsT=wq_sb[:, d, h * HD:(h + 1) * HD],
                        rhs=x_sb[:, d, csl],
                        start=(d == 0), stop=(d == DT - 1),
                    )
                    if d == DT - 1:
                        nc.vector.tensor_copy(qT_sb[:, h, csl], pq)
                return emit

            return [unit(d) for d in range(DT)]

        out_pr = outT.rearrange("(ot p) s -> p ot s", p=P)   # [128, 16, 2048]

        def make_ph3_units(c, gp_copies=False):
            """Out-projection units for chunk c. Output tiles are paired
            ([P, 2, CH]) so each DMA call covers two ot tiles — halves the
            SWDGE issue cost — and DMA issue alternates sync/gpsimd.
            gp_copies routes the PSUM->SBUF copies to the idle gpsimd engine
            (used for the units consumed during the last chunk, where scalar
            is exp-bound and the DVE runs the denominator chain)."""
            csl = slice(c * CH, (c + 1) * CH)
            state = {}

            def unit(ot):
                def emit():
                    pout = ro_ps.tile([P, CH], f32, tag="rout", name=f"pout{c}_{ot}")
                    for di in range(HDC // P):
                        nc.tensor.matmul(
                            pout,
                            lhsT=wo_sb[:, di, ot * P:(ot + 1) * P],
                            rhs=oT_sb[:, di, csl],
                            start=(di == 0), stop=(di == HDC // P - 1),
                        )
                    if ot % 2 == 0:
                        state["ob"] = obpool.tile([P, 2, CH], f16, tag="ob", name=f"ob{c}_{ot}")
                    ob = state["ob"]
                    nc.vector.tensor_copy(ob[:, ot % 2, :], pout)
                    if ot % 2 == 1:
                        if gp_copies:
                            # sync only: scalar must stay exp-only and
                            # gpsimd's dge drain must not trail the end
                            eng = nc.sync
                        else:
                            eng = nc.gpsimd if (ot // 2) % 2 else nc.sync
                        eng.dma_start(out=out_pr[:, ot - 1:ot + 1, csl], in_=ob)
                return emit

            return [unit(ot) for ot in range(DT)]

        # ---------- Phase 2: attention with interleaved fillers ----------
        def attention(c, h, fillers, fills_per_g, prev_fin=None, last=False):
            """fillers: list of emission units injected after each score pair.
            Returns a finish-closure (den -> ones-matmul -> recip -> mul)
            which the NEXT attention invokes right after its first score
            pair, so the den-dependent ones-matmul never blocks the next
            iteration's independent scores in the in-order PE queue."""
            csl = slice(c * CH, (c + 1) * CH)
            po = o_ps.tile([P, CH], f32, tag="po", name=f"po{c}_{h}")
            den2 = dnpool.tile([P, 2, CH], f16, tag="den", name=f"den2{c}_{h}")
            den = dnpool.tile([P, CH], f16, tag="den", name=f"den{c}_{h}")
            fi = 0
            pt0 = None
            pt_late = []

            def scores(g):
                psc = sc_ps.tile([P, 2, CH], f32, tag="psc", name=f"psc{c}_{h}_{g}")
                for j in range(2):
                    kj = g * 2 + j
                    nc.tensor.matmul(
                        psc[:, j, :],
                        lhsT=kT_sb[:, h, kj * P:(kj + 1) * P],
                        rhs=qT_sb[:, h, csl],
                        start=True, stop=True,
                    )
                return psc

            # software-pipelined by one group: scores(g+1) are emitted
            # BEFORE the exp(g)-dependent PV(g), so the in-order PE streams
            # them while the scalar engine runs exp(g). Without the
            # lookahead, exp(g+1) transitively waits on exp(g)'s PV and the
            # group cycle degrades to exp + ~950ns instead of max(exp, PE).
            psc = scores(0)
            if prev_fin is not None:
                prev_fin()
            for g in range(KT // 2):
                psc_next = scores(g + 1) if g + 1 < KT // 2 else None
                # independent PE work here hides the exp latency
                for _ in range(fills_per_g[g]):
                    if fi < len(fillers):
                        fillers[fi]()
                        fi += 1
                pt = ppool.tile([P, 2, CH], f16, tag="pt", name=f"pt{c}_{h}_{g}")
                nc.scalar.activation(
                    out=pt, in_=psc,
                    func=mybir.ActivationFunctionType.Exp, scale=SCALE,
                )
                for j in range(2):
                    kj = g * 2 + j
                    nc.tensor.matmul(
                        po,
                        lhsT=v_sb[:, kj, h * HD:(h + 1) * HD],
                        rhs=pt[:, j, :],
                        start=(kj == 0), stop=(kj == KT - 1),
                    )
                psc = psc_next
                # f16 pairwise accumulation of the exp'd tiles for the
                # softmax denominator: one [P, 2, CH] add per group (the DVE
                # processes 2 f16/cycle, so this halves both the op count
                # and the queue entries vs per-half adds). For the LAST
                # iteration, groups 6/7 skip the DVE and are folded into the
                # ones-matmul instead: those accumulates are gated on
                # exp(g6)/exp(g7) and land exactly in the final den window,
                # keeping the PE's p-state up (a ~2us idle there demotes the
                # clock to 1.2GHz for the whole out-projection sweep).
                if g == 0:
                    pt0 = pt
                elif g == 1:
                    nc.vector.tensor_add(den2, pt0, pt)
                elif last and g >= KT // 2 - 2:
                    pt_late.append(pt)
                else:
                    nc.vector.tensor_add(den2, den2, pt)
            while fi < len(fillers):
                fillers[fi]()
                fi += 1

            def finish():
                with tc.high_priority():
                    nc.vector.tensor_add(den, den2[:, 0, :], den2[:, 1, :])
                    pden = ro_ps.tile([P, CH], f32, tag="rout", name=f"pden{c}_{h}")
                    nc.tensor.matmul(pden, lhsT=ones, rhs=den,
                                     start=True, stop=(not pt_late))
                    for li, ptl in enumerate(pt_late):
                        for j in range(2):
                            nc.tensor.matmul(
                                pden, lhsT=ones, rhs=ptl[:, j, :], start=False,
                                stop=(li == len(pt_late) - 1 and j == 1),
                            )
                    rs = rspool.tile([P, CH], f32, tag="rs", name=f"rs{c}_{h}")
                    nc.vector.reciprocal_approx_fast(out=rs, in_=pden)
                    nc.vector.tensor_mul(oT_sb[:, h, csl], po, rs)

            return finish

        # Final-block helpers: the last chunk's out-projection accumulates
        # di=0 (head 0, available early) and di=1 (head 1, after the last
        # den chain) per ot tile in PSUM. 16 ot tiles flow through the 8
        # PSUM banks in two waves; wave-B banks are released by wave-A's
        # PSUM->SBUF pair-copies, which alternate between the scalar and
        # vector engines (both idle by then) so neither serializes the
        # drain. f16 pair tiles DMA out on sync/gpsimd.
        cl = slice((NCH - 1) * CH, NCH * CH)
        fpout = {}
        fptile = {}

        def falloc_pair(pi):
            t = sc_ps.tile([P, 2, CH], f32, tag="psc", name=f"fp{pi}")
            fptile[pi] = t
            fpout[2 * pi] = t[:, 0, :]
            fpout[2 * pi + 1] = t[:, 1, :]

        def falloc_single(ot, pool, tag):
            fpout[ot] = pool.tile([P, CH], f32, tag=tag, name=f"fs{ot}")

        def fdi0(ot):
            nc.tensor.matmul(
                fpout[ot], lhsT=wo_sb[:, 0, ot * P:(ot + 1) * P],
                rhs=oT_sb[:, 0, cl], start=True, stop=False,
            )

        def fdi1(ot):
            nc.tensor.matmul(
                fpout[ot], lhsT=wo_sb[:, 1, ot * P:(ot + 1) * P],
                rhs=oT_sb[:, 1, cl], start=False, stop=True,
            )

        def fpair_unit(pi):
            def emit():
                falloc_pair(pi)
                fdi0(2 * pi)
                fdi0(2 * pi + 1)
            return emit

        def fot4_unit():
            # single on o_ps: aliases po(c3,h0), long freed by h0's mul
            falloc_single(4, o_ps, "po")
            fdi0(4)

        # Filler hosting, balanced so no attention iteration's PE work
        # (scores+PV = ~0.86us/group) falls below the exp pace
        # (~1.1us/group): chunk-0 attention hosts TWO deferred-q sets
        # (chunks 1 and 2), chunk-1 hosts q(3) plus chunk-0's out-proj,
        # chunks 2/3 host the previous chunk's out-proj.
        fin = None
        for c in range(NCH):
            ph3_prev = make_ph3_units(c - 1, gp_copies=(c == NCH - 1)) if c > 0 else []
            for h in range(HPC):
                ph = ph3_prev[h * 8:(h + 1) * 8]
                if c == 0:
                    fills = make_defq_units(1, h) + make_defq_units(2, h)
                    pat = [4, 4, 4, 4, 4, 4, 4, 0]
                elif c == 1:
                    fills = make_defq_units(3, h) + ph
                    pat = [4, 4, 3, 3, 0, 2, 2, 0]
                elif c == 2:
                    fills = ph
                    pat = [1, 1, 1, 1, 1, 1, 1, 1]
                else:
                    fills = ph
                    if h == HPC - 1:
                        # di=0 units for ot 0-4 of c3's own out-projection ride
                        # as post-loop fillers of the LAST attention so they
                        # land in the PE queue BEFORE the den-dependent
                        # ones-matmul and fill the denominator-chain window.
                        # (only ONE ro-slot consumer may precede pden in the
                        # ro ring — none here — else it deadlocks)
                        fills = ph + [fpair_unit(0), fpair_unit(1), fot4_unit]
                    pat = [1, 1, 1, 1, 1, 0, 0, 0]
                fin = attention(c, h, fills, pat, prev_fin=fin,
                                last=(c == NCH - 1 and h == HPC - 1))
        fin()

        # di=0 staircase over the banks freed by the last den chain:
        # ot6 (ro slot freed by chunk-2 pout cast) -> ot7 (pden's slot,
        # freed by the reciprocal) -> ot5 (po(c3,h1), freed by the mul)
        falloc_single(6, ro_ps, "rout"); fdi0(6)
        falloc_single(7, ro_ps, "rout"); fdi0(7)
        falloc_single(5, o_ps, "po"); fdi0(5)

        def fdrain(pi):
            """Copy pair pi's two f32 PSUM ot tiles to an f16 pair and DMA.
            Wide single-op casts for contiguous sc pairs; per-half casts on
            both engines otherwise. Copy engine alternates scalar/vector."""
            ob = obpool.tile([P, 2, CH], f16, tag="ob", name=f"fob{pi}")
            if pi == DT // 2 - 1:
                # last pair: per-half copies and DMAs so the final flush
                # starts as early as possible
                nc.scalar.copy(ob[:, 0, :], fpout[2 * pi])
                nc.sync.dma_start(out=out_pr[:, 2 * pi, cl], in_=ob[:, 0, :])
                nc.vector.tensor_copy(ob[:, 1, :], fpout[2 * pi + 1])
                nc.scalar.dma_start(out=out_pr[:, 2 * pi + 1, cl], in_=ob[:, 1, :])
                return
            if pi in fptile:
                if pi % 2:
                    nc.vector.tensor_copy(ob, fptile[pi])
                else:
                    nc.scalar.copy(ob, fptile[pi])
            else:
                e0, e1 = (nc.scalar, nc.vector) if pi % 2 == 0 else (nc.vector, nc.scalar)
                (e0.copy if e0 is nc.scalar else e0.tensor_copy)(ob[:, 0, :], fpout[2 * pi])
                (e1.copy if e1 is nc.scalar else e1.tensor_copy)(ob[:, 1, :], fpout[2 * pi + 1])
            # alternate sync/scalar (scalar's exp work is over by now and a
            # sync-only chain leaves the kernel waiting ~2us on the last
            # serialized transfers); gpsimd's dge drain must not trail the
            # kernel end, so it carries nothing here
            (nc.sync if pi % 2 == 0 else nc.scalar).dma_start(
                out=out_pr[:, 2 * pi:2 * pi + 2, cl], in_=ob)

        for ot in range(8):
            fdi1(ot)
            if ot % 2 == 1:
                fdrain(ot // 2)
        # wave B: banks released by wave-A pair-copy completions
        for ot in range(8, DT):
            if ot % 2 == 0 and ot < 12:
                falloc_pair(ot // 2)
            elif ot == 12:
                falloc_single(12, o_ps, "po")
            elif ot == 13:
                falloc_single(13, o_ps, "po")
            elif ot == 14:
                falloc_single(14, ro_ps, "rout")
            elif ot == 15:
                falloc_single(15, ro_ps, "rout")
            fdi0(ot)
            fdi1(ot)
            if ot % 2 == 1:
                fdrain(ot // 2)

    nc.compile()
    return nc


def _get_nc():
    if "nc" not in _CACHE:
        _CACHE["nc"] = _build_nc()
    return _CACHE["nc"]


def make_in_maps(x, w_q, w_k, w_v, w_o):
    x = np.asarray(x, dtype=np.float32).reshape(S, D)
    w_q = np.asarray(w_q, dtype=np.float32)
    w_k = np.asarray(w_k, dtype=np.float32)
    w_v = np.asarray(w_v, dtype=np.float32)
    w_o = np.asarray(w_o, dtype=np.float32)
    xT = np.ascontiguousarray(x.T).astype(np.float16)
    in_maps = []
    for c in range(NCORES):
        hs = slice(c * HDC, (c + 1) * HDC)
        in_maps.append({
            "xT": xT,
            "wqT": np.ascontiguousarray(w_q[hs, :].T).astype(np.float16),
            "wkT": np.ascontiguousarray(w_k[hs, :].T).astype(np.float16),
            "wvT": np.ascontiguousarray(w_v[hs, :].T).astype(np.float16),
            "woT": np.ascontiguousarray(w_o[:, hs].T).astype(np.float16),
        })
    return in_maps


def kernel(x, w_q, w_k, w_v, w_o):
    global LAST_RESULT
    in_maps = make_in_maps(x, w_q, w_k, w_v, w_o)
    nc = _get_nc()
    res = run_bass_kernel_spmd(nc, in_maps, core_ids=list(range(NCORES)))
    LAST_RESULT = res
    acc = np.zeros((D, S), dtype=np.float32)
    for r in res.results:
        acc += r["outT"].astype(np.float32)
    return np.ascontiguousarray(acc.T).astype(np.float32).reshape(1, S, D)

